# revision 17
# baseline (speedup 1.0000x reference)
"""Trainium2 Bass kernel for fused multi-head attention (dense transformer).

y = proj(softmax(QK^T/sqrt(d)) V) for x [4, 2048, 512], 16 heads, d=32.

Sharding (8 cores): core c handles batch b = c//2 and head-group hg = c%2
(8 heads each); the two half-results per batch are summed on the host.

Feature-major on-chip layouts; all attention matmuls run fp8e4m3 with
DoubleRow (0.5 PE cycles/row); q/k/v weights pre-scaled by 64 host-side.

Key structure (vs the v0 kernel this replaces):
  qT/kT [128, 2g, 2i, T] fp8: partition p = 32*(h%4)+d -> QK psum evictions
       run at full 128 partitions (16 ops of 512 cols instead of 32).
  vz   [128, 8ktp, 2i, 2e, 4pr, 128] fp8 zero-padded per-head AV windows:
       head h = 2*pr+e has V*64 at window cols [32e,32e+32), a 1.0
       denominator column at col 64+32e, zeros elsewhere. Both heads of a
       pair accumulate into ONE psum bank: O_h0 rows 0:32, O_h1 rows
       32:64, sums rows 64 and 96 (32-aligned for engine access) -- one
       accumulation group, same-slice writes keep PE order.
  norm per PAIR (not per head): one [97,512] psum->SBUF evict, two [1,512]
       reciprocals, two Pool partition_broadcasts, one mul -> oT8 bf16.
  exp  [128, 2, 512] tiles balanced greedily between ACT (native Exp) and
       DVE (1-op Schraudolph: A*s+B -> rint -> int8 bitcast as fp8).
       Pool has no PSUM port so it only gets SBUF work (muls, memsets).

PSUM: sT tag [128,2,512] x3 bufs = 6 banks (also QK-prep + proj psum);
oP0/oP1 1 bank each (pair accumulators; also V-prep psum).
"""

import os
import sys

sys.path.insert(0, "/opt/trn_rl_repo")

import numpy as np
import ml_dtypes

F8 = ml_dtypes.float8_e4m3
BF16 = ml_dtypes.bfloat16

N_CORES = 8
B, T, C = 4, 2048, 512
N_HEADS, HEAD_DIM = 16, 32
HPC = 8            # heads per core
SCALE = 1.0 / np.sqrt(np.float32(HEAD_DIM))
W_SCALE = 64.0     # host-side weight prescale (exact power of 2)
QK_FACT = W_SCALE * W_SCALE  # 4096: scores come out multiplied by this
# Schraudolph-in-fp8: bits = rint(s * A8 + B8); bitcast int8 -> fp8e4m3
A8 = 2.0403 / 4096.0
B8 = 55.625

QC = 4             # q chunks of 512
KT = 16            # k tiles of 128
KTP = 8            # k tile pairs
TT = 16            # t tiles of 128 (for V)
NPAIR = 4          # head pairs per core

_CACHE = {}


def _build():
    import concourse.bass as bass
    import concourse.tile as tile
    from concourse import bacc, mybir
    from concourse.tile_rust import add_dep_helper

    f32 = mybir.dt.float32
    f8 = mybir.dt.float8e4
    i8 = mybir.dt.int8
    bf16 = mybir.dt.bfloat16
    Exp = mybir.ActivationFunctionType.Exp
    Identity = mybir.ActivationFunctionType.Identity
    Copy = mybir.ActivationFunctionType.Copy
    Recip = mybir.ActivationFunctionType.Reciprocal
    DR = mybir.MatmulPerfMode.DoubleRow
    MUL = mybir.AluOpType.mult
    ADD = mybir.AluOpType.add
    ts = bass.ts
    ds = bass.ds

    nc = bacc.Bacc("TRN2", target_bir_lowering=False, debug=False,
                   num_devices=N_CORES)

    x8_d = nc.dram_tensor("xb", (128, 4, T), bf16, kind="ExternalInput")
    xf_d = nc.dram_tensor("xf", (128, 2, 2, T), f8, kind="ExternalInput")
    wq_d = nc.dram_tensor("wqf", (128, 2, 2, 256), f8, kind="ExternalInput")
    wk_d = nc.dram_tensor("wkf", (128, 2, 2, 256), f8, kind="ExternalInput")
    wv_d = nc.dram_tensor("wvb", (128, 4, 256), bf16, kind="ExternalInput")
    bq_d = nc.dram_tensor("bq", (128, 2), f32, kind="ExternalInput")
    bk_d = nc.dram_tensor("bk", (128, 2), f32, kind="ExternalInput")
    wp_d = nc.dram_tensor("wpb", (128, 2, C), bf16, kind="ExternalInput")
    outT_d = nc.dram_tensor("outT", (C, T), f32, kind="ExternalOutput")
    scr_d = nc.dram_tensor("scrR", (QC * NPAIR, 2, 512), f32,
                           kind="Internal")

    from contextlib import ExitStack

    # greedy psum-engine balancer (estimated busy ns per engine)
    bal = {"act": 0.0, "dve": 0.0}

    def c_act(free):
        return 0.8333 * free + 185.0

    def c_dve(free):
        return 1.0417 * free + 125.0

    def pick(free):
        if bal["act"] + c_act(free) <= bal["dve"] + c_dve(free):
            bal["act"] += c_act(free)
            return "act"
        bal["dve"] += c_dve(free)
        return "dve"

    with tile.TileContext(nc) as tc, ExitStack() as ctx:
        persist = ctx.enter_context(tc.tile_pool(name="persist", bufs=1))
        psum = ctx.enter_context(tc.tile_pool(name="psum", bufs=1,
                                              space="PSUM"))
        pTp = ctx.enter_context(tc.tile_pool(name="pTp", bufs=10))
        oSp = ctx.enter_context(tc.tile_pool(name="oSp", bufs=6))
        r2p = ctx.enter_context(tc.tile_pool(name="r2p", bufs=4))
        rrp = ctx.enter_context(tc.tile_pool(name="rrp", bufs=4))
        ostage = ctx.enter_context(tc.tile_pool(name="ostage", bufs=4))

        # ---- persistent SBUF ----
        x8 = persist.tile([128, 4, T], bf16)
        xf = persist.tile([128, 2, 2, T], f8)
        wq8 = persist.tile([128, 2, 2, 256], f8)
        wk8 = persist.tile([128, 2, 2, 256], f8)
        wv8 = persist.tile([128, 4, 256], bf16)
        wp8 = persist.tile([128, 2, C], bf16)
        bq = persist.tile([128, 2], f32)
        bk = persist.tile([128, 2], f32)
        # qT/kT: partition p = 32*(h%4)+d, dims (g=h//4, i, t)
        qT = persist.tile([128, 2, 2, T], f8)
        kT = persist.tile([128, 2, 2, T], f8)
        # vz[p, ktp, i, e, pr, w]: zero-padded AV window for head h=2*pr+e
        vz = persist.tile([128, KTP, 2, 2, NPAIR, 128], f8)
        oT8 = persist.tile([128, 2, T], bf16)

        # ---- loads (x8 chunk 0 + qk weights + biases first) ----
        nc.sync.dma_start(wq8[:], wq_d.ap())
        nc.sync.dma_start(wk8[:], wk_d.ap())
        nc.sync.dma_start(xf[:, :, :, ts(0, 512)],
                          xf_d.ap()[:, :, :, ts(0, 512)])
        nc.sync.dma_start(bq[:], bq_d.ap())
        nc.sync.dma_start(bk[:], bk_d.ap())
        nc.sync.dma_start(wv8[:], wv_d.ap())
        for tch in range(1, QC):
            nc.sync.dma_start(xf[:, :, :, ts(tch, 512)],
                              xf_d.ap()[:, :, :, ts(tch, 512)])
        for tch in range(QC):
            nc.sync.dma_start(x8[:, :, ts(tch, 512)],
                              x8_d.ap()[:, :, ts(tch, 512)])
        nc.sync.dma_start(wp8[:], wp_d.ap())

        # ---- constant regions (Pool) ----
        # DR i=1 slots must be finite: kT i=1 must be 0 (zero-pad trick),
        # qT i=1 finite (0*x would be 0*NaN = NaN otherwise)
        nc.gpsimd.memset(kT[:, :, 1, :], 0.0)
        nc.gpsimd.memset(qT[:, :, 1, :], 0.0)
        nc.gpsimd.memset(vz[:], 0.0)
        for e in range(2):
            nc.gpsimd.memset(vz[:, :, :, e, :, 64 + 32 * e:65 + 32 * e], 1.0)

        def sT_tile():
            return psum.tile([128, 2, 512], f32, tag="sT", bufs=3,
                             name="sT")

        def oP_tile(i):
            return psum.tile([128, 512], f32, tag=f"oP{i}", name=f"oP{i}")

        def emit_evict_qk(dst_ap, src_ap, bias_ap):
            if pick(512) == "act":
                nc.scalar.activation(dst_ap, src_ap, Identity, bias=bias_ap)
            else:
                nc.vector.tensor_scalar_add(dst_ap, src_ap, bias_ap)

        def emit_copy(dst_ap, src_ap, free, scale=None):
            if pick(free) == "act":
                if scale is None:
                    nc.scalar.activation(dst_ap, src_ap, Copy)
                else:
                    nc.scalar.activation(dst_ap, src_ap, Copy, scale=scale)
            else:
                if scale is None:
                    nc.vector.tensor_copy(dst_ap, src_ap)
                else:
                    nc.vector.tensor_scalar_mul(dst_ap, src_ap, scale)

        def emit_exp(pT_ap, pT_i8_ap, sT_ap):
            if pick(1024) == "act":
                nc.scalar.activation(pT_ap, sT_ap, Exp,
                                     scale=float(SCALE) / QK_FACT)
            else:
                nc.vector.tensor_scalar(pT_i8_ap, sT_ap, A8, B8, MUL, ADD)

        # ---- prep: QKV projections ----
        def emit_qk_tch(tch):
            for g in range(2):
                pt = sT_tile()
                for qk, w8 in enumerate((wq8, wk8)):
                    for a in range(2):
                        nc.tensor.matmul(
                            pt[:, qk, :],
                            w8[:, a, :, ds(128 * g, 128)],
                            xf[:, a, :, ts(tch, 512)],
                            start=(a == 0), stop=(a == 1),
                            perf_mode=DR,
                        )
                # k first: attention needs the full kT before it can start
                for qk, (bt, dT) in ((1, (bk, kT)), (0, (bq, qT))):
                    emit_evict_qk(dT[:, g, 0, ts(tch, 512)],
                                  pt[:, qk, :], bt[:, g:g + 1])

        def emit_v_tt(tt):
            vt = oP_tile(tt % 2)
            for kc in range(4):
                nc.tensor.matmul(
                    vt[:, 0:256],
                    x8[:, kc, ts(tt, 128)],
                    wv8[:, kc, :],
                    start=(kc == 0), stop=(kc == 3),
                )
            for e in range(2):
                src = vt[:, ds(128 * e, 128)].rearrange(
                    "p (r d) -> p r d", r=4)
                dst = vz[:, tt // 2, tt % 2, e, :, ds(32 * e, 32)]
                if pick(128) == "act":
                    nc.scalar.activation(dst, src, Copy)
                else:
                    nc.vector.tensor_copy(dst, src)

        # all of kT/qT first (attention needs full kT), then V-prep:
        # even tt claim oP0 generations ahead of pair 0, odd tt ahead of
        # pair 1 -- the scheduler overlaps V work with the first pairs' exps
        for tch in range(QC):
            emit_qk_tch(tch)
        # evens (oP0 generations) then odds (oP1): all claims precede the
        # pairs' accumulator claims; the scheduler overlaps the V work with
        # the first pairs' exp streams
        for tt in range(0, TT, 2):
            emit_v_tt(tt)
        for tt in range(1, TT, 2):
            emit_v_tt(tt)

        # ---- attention (pair-serial) ----
        def emit_pair(qc, pr, op, deferred=(), eager=False):
            dq = list(deferred)
            for e in range(2):
                h = 2 * pr + e
                g, m = h // 4, h % 4
                for ktp in range(KTP):
                    if dq and (eager or (e, ktp) in
                               ((0, 2), (0, 5), (1, 2), (1, 5))):
                        dq.pop(0)()
                    st = sT_tile()
                    for i in range(2):
                        kt = 2 * ktp + i
                        nc.tensor.matmul(
                            st[:, i, :],
                            kT[ds(32 * m, 32), g, :, ts(kt, 128)],
                            qT[ds(32 * m, 32), g, :, ts(qc, 512)],
                            start=True, stop=True, perf_mode=DR,
                            tile_position=(32 * m, 0),
                        )
                    pT = pTp.tile([128, 2, 512], f8, tag="pT", name="pT")
                    emit_exp(pT[:], pT[:].bitcast(i8), st[:])
                    nc.tensor.matmul(
                        op[:, :],
                        vz[:, ktp, :, e, pr, :],
                        pT[:],
                        start=(e == 0 and ktp == 0),
                        stop=(e == 1 and ktp == KTP - 1),
                        perf_mode=DR,
                    )
            while dq:
                dq.pop(0)()

        def emit_norm(qc, pr, op, fast=False):
            pid = qc * NPAIR + pr
            oS = oSp.tile([97, 512], f32, tag="oS", name="oS")
            emit_copy(oS[:], op[0:97, :], 512)
            rr = rrp.tile([64, 512], f32, tag="rr", name="rr")
            ra = r2p.tile([1, 512], f32, tag="ra", name="ra")
            rb = r2p.tile([1, 512], f32, tag="rb", name="rb")
            bal["dve"] += 2 * c_dve(512)
            nc.vector.reciprocal(ra[:], oS[64:65, :])
            nc.vector.reciprocal(rb[:], oS[96:97, :])
            sts = []
            sts.append(nc.sync.dma_start(scr_d.ap()[pid, 0:1, :], ra[:]))
            sts.append(nc.sync.dma_start(scr_d.ap()[pid, 1:2, :], rb[:]))
            for e in range(2):
                ld_i = nc.sync.dma_start(
                    rr[ds(32 * e, 32), :],
                    scr_d.ap()[pid, e:e + 1, :].squeeze(0).unsqueeze(0)
                    .broadcast_to((32, 512)))
                add_dep_helper(ld_i.ins, sts[e].ins, sync=True,
                               reason="scrR roundtrip load waits store")
            g = (2 * pr) // 4
            dst = oT8[ds(64 * (pr % 2), 64), g, ts(qc, 512)]
            with nc.allow_low_precision(reason="softmax norm fp8-grade"):
                nc.gpsimd.tensor_tensor(dst, oS[0:64, :], rr[:], MUL)

        def emit_proj_ct(qc, ct):
            # borrow an oP bank: keeps the sT rotation (the exp pipeline)
            # free of the long norm->DMA->oT8 dependency chain
            pps = oP_tile(ct % 2)
            for yt in range(2):
                nc.tensor.matmul(
                    pps[:, :], wp8[:, yt, ts(ct, 128)],
                    oT8[:, yt, ts(qc, 512)],
                    start=(yt == 0), stop=(yt == 1),
                )
            ost = ostage.tile([128, 512], f32, tag="ost", name="ost")
            emit_copy(ost[:], pps[:, :], 512, scale=1.0 / W_SCALE)
            nc.sync.dma_start(
                outT_d.ap()[ts(ct, 128), ts(qc, 512)], ost[:])

        from collections import deque
        pending = deque()

        def make_norm(qc, pr, op):
            def f():
                emit_norm(qc, pr, op, fast=(qc == QC - 1))
            return f

        def make_proj(qc, ct):
            def f():
                emit_proj_ct(qc, ct)
            return f

        for qc in range(QC):
            for pr in range(NPAIR):
                op = oP_tile(pr % 2)
                eager = qc == QC - 1 and pr >= 2
                todo = []
                for _ in range(8 if eager else 2):
                    if pending:
                        todo.append(pending.popleft())
                emit_pair(qc, pr, op, deferred=todo, eager=eager)
                pending.append(make_norm(qc, pr, op))
            if qc < QC - 1:
                for ct in range(4):
                    pending.append(make_proj(qc, ct))
        while pending:
            pending.popleft()()
        for ct in range(4):
            emit_proj_ct(QC - 1, ct)

    nc.compile()
    nc._engine_balance = dict(bal)
    return nc


def _get_nc():
    if "nc" not in _CACHE:
        _CACHE["nc"] = _build()
    return _CACHE["nc"]


def kernel(x, w_attn, b_attn, w_proj, b_proj):
    from concourse.bass_utils import run_bass_kernel_spmd

    x = np.asarray(x, dtype=np.float32)
    w_attn = np.asarray(w_attn, dtype=np.float32)
    b_attn = np.asarray(b_attn, dtype=np.float32)
    w_proj = np.asarray(w_proj, dtype=np.float32)
    b_proj = np.asarray(b_proj, dtype=np.float32)

    nc = _get_nc()

    # e-major column permutation for wv: head h=2*pr+e, dim d -> 128e+32pr+d
    vperm = np.empty(256, dtype=np.int64)
    for h in range(HPC):
        pr, e = h // 2, h % 2
        vperm[128 * e + 32 * pr + np.arange(32)] = 32 * h + np.arange(32)

    in_maps = []
    for core in range(N_CORES):
        b, hg = core // 2, core % 2
        cs = hg * 256
        x8 = np.ascontiguousarray(
            x[b].T.reshape(4, 128, T).transpose(1, 0, 2)).astype(BF16)
        xf = np.ascontiguousarray(
            x[b].T.reshape(2, 2, 128, T).transpose(2, 0, 1, 3)).astype(F8)

        def wpack(w):  # [C, 256] -> [128, 4, 256], scaled, bf16
            return np.ascontiguousarray(
                (w * W_SCALE).reshape(4, 128, 256)
                .transpose(1, 0, 2)).astype(BF16)

        def wpack8(w):  # [C, 256] -> [128, 2, 2, 256], scaled, fp8
            return np.ascontiguousarray(
                (w * W_SCALE).reshape(2, 2, 128, 256)
                .transpose(2, 0, 1, 3)).astype(F8)

        wq8 = wpack8(w_attn[:, cs:cs + 256])
        wk8 = wpack8(w_attn[:, C + cs:C + cs + 256])
        wv8 = wpack(w_attn[:, 2 * C + cs:2 * C + cs + 256][:, vperm])
        bq = np.ascontiguousarray(
            (b_attn[cs:cs + 256] * W_SCALE).reshape(2, 128).T)
        bk = np.ascontiguousarray(
            (b_attn[C + cs:C + cs + 256] * W_SCALE).reshape(2, 128).T)
        wp8 = np.ascontiguousarray(
            w_proj[cs:cs + 256, :].reshape(2, 128, C)
            .transpose(1, 0, 2)).astype(BF16)
        in_maps.append({
            "xb": x8, "xf": xf, "wqf": wq8, "wkf": wk8, "wvb": wv8,
            "bq": bq.astype(np.float32), "bk": bk.astype(np.float32),
            "wpb": wp8,
        })

    res = run_bass_kernel_spmd(nc, in_maps, core_ids=list(range(N_CORES)))

    b_eff = (b_proj + b_attn[2 * C:3 * C] @ w_proj).astype(np.float32)
    out = np.empty((B, T, C), dtype=np.float32)
    for b in range(B):
        acc = res.results[2 * b]["outT"].T + res.results[2 * b + 1]["outT"].T
        out[b] = acc + b_eff
    return out


if __name__ == "__main__":
    rng = np.random.default_rng(0)
    x = rng.standard_normal((B, T, C), dtype=np.float32)
    w_attn = rng.standard_normal((C, 3 * C), dtype=np.float32) * 0.02
    b_attn = rng.standard_normal(3 * C, dtype=np.float32) * 0.02
    w_proj = rng.standard_normal((C, C), dtype=np.float32) * 0.02
    b_proj = rng.standard_normal(C, dtype=np.float32) * 0.02
    out = kernel(x, w_attn, b_attn, w_proj, b_proj)
    print("kernel out", out.shape, out.dtype, float(np.abs(out).max()))
